# revision 1
# baseline (speedup 1.0000x reference)
"""Multi-head attention (B=2, S=2048, D=1024, H=16) on 8 Trainium2 cores.

Sharding: 2-way data parallel on batch x 4-way tensor parallel on heads.
Core c handles batch b = c // 4 and head group g = c % 4 (4 heads, 256 dims).

Per-core pipeline (all "feature-major" / transposed layouts so that every
matmul streams a long free dim and biases land on partitions):
  XT [1024, 2048]  (host-transposed input slice, bf16)
  QT = (Wq.T @ X.T + bq) / 8   [256, 2048]
  KT =  Wk.T @ X.T + bk        [256, 2048]
  V  =  X @ Wv + bv            [2048, 256]   (bias via ones-row matmul)
  per head h:
    scoresT[t, s] = KT_h[:, t]_tile.T @ QT_h      (PE, K=64)
    expT = exp(scoresT)                           (ACT, PSUM -> SBUF bf16)
    hoT'[65, s]  = [V_h | 1].T @ expT             (PE, accumulate over t)
      rows 0:64 = unnormalized head out (d, s), row 64 = sumexp[s]
    hoT = hoT' / sumexp                           (GpSimd bcast + DVE)
  outT[e, s] = Wout_g.T-ish: lhsT=Wout_g tiles, rhs=hoT  (partial over group)
Host sums the 4 per-group partials per batch and transposes back.
"""

import numpy as np
import ml_dtypes

BF16 = ml_dtypes.bfloat16

S = 2048  # sequence length
C = 1024  # d_model
NH = 16  # total heads
DK = 64  # head dim
N_CORES = 8
HPC = 4  # heads per core
DH = HPC * DK  # 256 per-core head dims
P = 128
VW = 72  # per-head stride in the V' buffer: 64 V cols + 1 ones col + 7 pad

_CACHE = {}


def _build_program():
    import concourse.bacc as bacc
    import concourse.mybir as mybir
    import concourse.tile as tile
    from contextlib import ExitStack

    dt = mybir.dt
    AF = mybir.ActivationFunctionType

    nc = bacc.Bacc("TRN2", target_bir_lowering=False, debug=False,
                   num_devices=N_CORES)

    xt = nc.dram_tensor("xt", [C, S], dt.bfloat16, kind="ExternalInput")
    wq = nc.dram_tensor("wq", [C, DH], dt.bfloat16, kind="ExternalInput")
    wk = nc.dram_tensor("wk", [C, DH], dt.bfloat16, kind="ExternalInput")
    wv = nc.dram_tensor("wv", [C, DH], dt.bfloat16, kind="ExternalInput")
    wo = nc.dram_tensor("wo", [DH, C], dt.bfloat16, kind="ExternalInput")
    # cols: [bq_tile0/8, bq_tile1/8, bk_tile0, bk_tile1]
    bqk = nc.dram_tensor("bqk", [P, 4], dt.float32, kind="ExternalInput")
    bv = nc.dram_tensor("bv", [1, DH], dt.bfloat16, kind="ExternalInput")
    outT = nc.dram_tensor("outT", [C, S], dt.bfloat16, kind="ExternalOutput")

    xt_r = xt.rearrange("(n p) s -> n p s", p=P)  # [8, 128, 2048]
    wq_r = wq.rearrange("(n p) d -> n p d", p=P)  # [8, 128, 256]
    wk_r = wk.rearrange("(n p) d -> n p d", p=P)
    wv_r = wv.rearrange("(n p) d -> n p d", p=P)
    wo_r = wo.rearrange("(n p) e -> n p e", p=P)  # [2, 128, 1024]
    outT_r = outT.rearrange("(n p) s -> n p s", p=P)  # [8, 128, 2048]

    with ExitStack() as ctx:
        tc = ctx.enter_context(tile.TileContext(nc))
        sb = ctx.enter_context(tc.tile_pool(name="sb", bufs=1))
        xpool = ctx.enter_context(tc.tile_pool(name="xpool", bufs=1))
        pool2 = ctx.enter_context(tc.tile_pool(name="pool2", bufs=1))
        spool = ctx.enter_context(tc.tile_pool(name="spool", bufs=2, space="PSUM"))
        vpool = ctx.enter_context(tc.tile_pool(name="vpool", bufs=1, space="PSUM"))

        # ---- persistent SBUF ----
        qt_sb = [sb.tile([P, S], dt.bfloat16, name=f"qt{i}", tag=f"qt{i}") for i in range(2)]
        kt_sb = [sb.tile([P, S], dt.bfloat16, name=f"kt{i}", tag=f"kt{i}") for i in range(2)]
        v_sb = [sb.tile([P, HPC * VW], dt.bfloat16, name=f"v{i}", tag=f"v{i}") for i in range(16)]
        exp_sb = [sb.tile([P, S], dt.bfloat16, name=f"e{i}", tag=f"e{i}") for i in range(16)]
        hot_sb = [sb.tile([P, S], dt.bfloat16, name=f"ho{i}", tag=f"ho{i}") for i in range(2)]
        wo_sb = [sb.tile([P, C], dt.bfloat16, name=f"wo{i}", tag=f"wo{i}") for i in range(2)]
        bqk_sb = sb.tile([P, 4], dt.float32, name="bqk", tag="bqk")
        bv_sb = sb.tile([1, DH], dt.bfloat16, name="bv", tag="bv")
        ones_sb = sb.tile([1, P], dt.bfloat16, name="ones", tag="ones")

        # ---- phase-1-only SBUF ----
        xt_sb = [xpool.tile([P, S], dt.bfloat16, name=f"x{i}", tag=f"x{i}") for i in range(8)]
        wq_sb = [xpool.tile([P, DH], dt.bfloat16, name=f"wq{i}", tag=f"wq{i}") for i in range(8)]
        wk_sb = [xpool.tile([P, DH], dt.bfloat16, name=f"wk{i}", tag=f"wk{i}") for i in range(8)]
        wv_sb = [xpool.tile([P, DH], dt.bfloat16, name=f"wv{i}", tag=f"wv{i}") for i in range(8)]

        # ---- loads: xt on the two HWDGE queues (sync + scalar), weights on
        # the gpsimd software DGE so no compute engine burns time on DMA
        # issue mid-phase. xt comes in s-column halves so the first QKT
        # chunk can start after half the transfer ----
        for half in range(2):
            cs = slice(half * 1024, (half + 1) * 1024)
            for i in range(8):
                eng = nc.sync if i % 2 == 0 else nc.scalar
                eng.dma_start(out=xt_sb[i][:, cs], in_=xt_r[i][:, cs])
        for i in range(8):
            nc.gpsimd.dma_start(out=wq_sb[i], in_=wq_r[i])
        nc.gpsimd.dma_start(out=bqk_sb, in_=bqk[:, :])
        nc.gpsimd.dma_start(out=bv_sb, in_=bv[:, :])
        for i in range(8):
            nc.gpsimd.dma_start(out=wk_sb[i], in_=wk_r[i])
        for i in range(8):
            nc.gpsimd.dma_start(out=wv_sb[i], in_=wv_r[i])
        for i in range(2):
            nc.sync.dma_start(out=wo_sb[i], in_=wo_r[i])
        nc.vector.memset(ones_sb, 1.0)
        # pre-set the per-head ones column in each V' tile (cols h*VW + DK)
        for t in range(16):
            col = v_sb[t].rearrange("p (h w) -> p h w", w=VW)[:, :, DK:DK + 1]
            nc.vector.memset(col, 1.0)

        def qkt_unit(d2, ch, qk):
            """One [128, 1024] chunk of QT or KT for d-tile d2. The
            1/sqrt(dk) scale is folded into Wq host-side, so the epilogue is
            a plain bias-add on DVE (keeps ACT free for exp)."""
            dst, w_sb, bias_col = (
                (qt_sb, wq_sb, 0) if qk == 0 else (kt_sb, wk_sb, 2)
            )
            ps = spool.tile([P, 1024], dt.float32, name="mm", tag="mm")
            for half in range(2):
                for c8 in range(8):
                    nc.tensor.matmul(
                        ps[:, half * 512:(half + 1) * 512],
                        lhsT=w_sb[c8][:, d2 * P:(d2 + 1) * P],
                        rhs=xt_sb[c8][:, ch * 1024 + half * 512:
                                      ch * 1024 + (half + 1) * 512],
                        start=(c8 == 0), stop=(c8 == 7),
                    )
            nc.vector.tensor_scalar_add(
                dst[d2][:, ch * 1024:(ch + 1) * 1024], ps,
                bqk_sb[:, bias_col + d2:bias_col + d2 + 1],
            )

        def qkt_half(d2):
            for ch in range(2):  # ch outer chases the half-column xt DMAs
                for qk in range(2):
                    qkt_unit(d2, ch, qk)

        def vproj_t(t):
            # V tile t: [128, 256] + bias via ones-row; packed [64|1|pad] x4.
            # Ones columns were pre-set at startup; the epilogue is a single
            # strided DVE copy so PE stays the pacer.
            ps = spool.tile([P, DH], dt.float32, name="mm", tag="mm")
            for c8 in range(8):
                nc.tensor.matmul(
                    ps, lhsT=xt_sb[c8][:, t * P:(t + 1) * P],
                    rhs=wv_sb[c8], start=(c8 == 0), stop=False,
                )
            nc.tensor.matmul(ps, lhsT=ones_sb, rhs=bv_sb,
                             start=False, stop=True)
            dst = v_sb[t].rearrange("p (h w) -> p h w", w=VW)[:, :, 0:DK]
            src = ps.rearrange("p (h w) -> p h w", w=DK)
            nc.vector.tensor_copy(dst, src)

        def scores_t(h, t):
            half_idx = h // 2
            row0 = (h % 2) * DK
            kth = kt_sb[half_idx]
            qth = qt_sb[half_idx]
            for ch in range(2):
                ps = spool.tile([P, 1024], dt.float32, name="mm", tag="mm")
                for half in range(2):
                    s0 = ch * 1024 + half * 512
                    nc.tensor.matmul(
                        ps[:, half * 512:(half + 1) * 512],
                        lhsT=kth[row0:row0 + DK, t * P:(t + 1) * P],
                        rhs=qth[row0:row0 + DK, s0:s0 + 512],
                        start=True, stop=True,
                    )
                nc.scalar.activation(
                    exp_sb[t][:, ch * 1024:(ch + 1) * 1024], ps, AF.Exp
                )

        def scores(h):
            for t in range(16):
                scores_t(h, t)

        sumtmp = pool2.tile([1, S], dt.float32, name="sumtmp", tag="sumtmp")
        sr = pool2.tile([1, S], dt.float32, name="sr", tag="sr")
        rbc = pool2.tile([DK, S], dt.float32, name="rbc", tag="rbc")

        def attn_block(h):
            # attn_h @ [V | 1] interleaved per-t with scores of head h+1:
            # ACT stays saturated with exp work through the whole stream.
            # t OUTER on attnV so each exp tile is fully consumed after 4
            # back-to-back matmuls (releases the WAR for head h+1's exp
            # writes immediately -- no pipeline convoy). The 4 s-chunk
            # accumulators live in one 4-bank PSUM tile.
            half_idx = h // 2
            row0 = (h % 2) * DK
            pv = vpool.tile([DK + 1, S], dt.float32, name="av", tag="av")
            for t in range(16):
                for ch4 in range(4):
                    nc.tensor.matmul(
                        pv[:, ch4 * 512:(ch4 + 1) * 512],
                        lhsT=v_sb[t][:, h * VW:h * VW + DK + 1],
                        rhs=exp_sb[t][:, ch4 * 512:(ch4 + 1) * 512],
                        start=(t == 0), stop=(t == 15),
                    )
                if h + 1 < HPC:
                    scores_t(h + 1, t)
            # normalization, chunk-pipelined: sumexp row (PSUM partition 64)
            # -> partition 0, approx reciprocal (custom DVE op needs base
            # partition 0), broadcast across 64 partitions, then one wide
            # multiply straight out of PSUM into the bf16 hoT buffer.
            for ch4 in range(4):
                c = slice(ch4 * 512, (ch4 + 1) * 512)
                nc.vector.tensor_copy(sumtmp[:, c], pv[DK:DK + 1, c])
                nc.vector.reciprocal_approx_fast(sr[:, c], sumtmp[:, c])
                nc.gpsimd.partition_broadcast(rbc[:, c], sr[:, c])
            nc.vector.tensor_mul(
                hot_sb[half_idx][row0:row0 + DK, :], pv[:DK, :], rbc,
            )

        def outproj():
            for e in range(8):
                for ch in range(2):
                    ps = spool.tile([P, 1024], dt.float32, name="mm", tag="mm")
                    for half in range(2):
                        s0 = ch * 1024 + half * 512
                        for d2 in range(2):
                            nc.tensor.matmul(
                                ps[:, half * 512:(half + 1) * 512],
                                lhsT=wo_sb[d2][:, e * P:(e + 1) * P],
                                rhs=hot_sb[d2][:, s0:s0 + 512],
                                start=(d2 == 0), stop=(d2 == 1),
                            )
                    st = pool2.tile([P, 1024], dt.bfloat16, name="st",
                                    tag="st", bufs=3)
                    if (e + ch) % 2 == 0:
                        nc.vector.tensor_copy(st, ps)
                        nc.sync.dma_start(
                            out=outT_r[e][:, ch * 1024:(ch + 1) * 1024],
                            in_=st)
                    else:
                        nc.scalar.copy(st, ps)
                        nc.scalar.dma_start(
                            out=outT_r[e][:, ch * 1024:(ch + 1) * 1024],
                            in_=st)

        # Emission order: QKT half 0 first (paced by the xt DMA), then a
        # PE-dense stream [scores0 | V | QKT half 1] that puts exp work on
        # ACT as early as possible, then the attention blocks (scores of
        # head h+1 ride inside head h's attnV) so ACT never starves.
        qkt_half(0)
        for t in range(16):
            scores_t(0, t)
            vproj_t(t)
            if t % 4 == 0:
                u = t // 4
                qkt_unit(1, u // 2, u % 2)
        attn_block(0)
        attn_block(1)
        attn_block(2)
        attn_block(3)
        outproj()

    nc.compile()
    return nc


def _get_program():
    if "nc" not in _CACHE:
        _CACHE["nc"] = _build_program()
    return _CACHE["nc"]


def _shard_inputs(input, W_qkv, b_qkv, W_out):
    """Build the 8 per-core input maps (host-side shard + transpose + cast)."""
    in_maps = []
    xt_by_b = [
        np.ascontiguousarray(input[b].T).astype(BF16) for b in range(2)
    ]
    for core in range(N_CORES):
        b, g = divmod(core, HPC)
        cols = slice(g * DH, (g + 1) * DH)
        bq = (b_qkv[g * DH:(g + 1) * DH] / 8.0).astype(np.float32)
        bk = b_qkv[C + g * DH:C + (g + 1) * DH].astype(np.float32)
        bqk = np.stack([bq[:P], bq[P:], bk[:P], bk[P:]], axis=1)
        in_maps.append({
            "xt": xt_by_b[b],
            "wq": np.ascontiguousarray(W_qkv[:, cols] * 0.125).astype(BF16),
            "wk": np.ascontiguousarray(W_qkv[:, C:2 * C][:, cols]).astype(BF16),
            "wv": np.ascontiguousarray(W_qkv[:, 2 * C:][:, cols]).astype(BF16),
            "wo": np.ascontiguousarray(W_out[g * DH:(g + 1) * DH, :]).astype(BF16),
            "bqk": np.ascontiguousarray(bqk, dtype=np.float32),
            "bv": b_qkv[2 * C + g * DH:2 * C + (g + 1) * DH]
                  .astype(BF16).reshape(1, DH),
        })
    return in_maps


def kernel(input, W_qkv, b_qkv, W_out):
    from concourse.bass_utils import run_bass_kernel_spmd

    nc = _get_program()
    in_maps = _shard_inputs(
        np.asarray(input), np.asarray(W_qkv), np.asarray(b_qkv),
        np.asarray(W_out),
    )
    res = run_bass_kernel_spmd(nc, in_maps, core_ids=list(range(N_CORES)))
    out = np.zeros((2, S, C), dtype=np.float32)
    for core in range(N_CORES):
        b = core // HPC
        out[b] += np.asarray(res.results[core]["outT"]).astype(np.float32).T
    return out



# revision 10
# speedup vs baseline: 1.0264x; 1.0264x over previous
"""Multi-head attention (B=2, S=2048, D=1024, H=16) on 8 Trainium2 cores.

Sharding: 2-way data parallel on batch x 4-way tensor parallel on heads.
Core c handles batch b = c // 4 and head group g = c % 4 (4 heads, 256 dims).

v2 design (ACT-bound pipeline):
  The exp over the per-head [S,S] score matrices is the per-core wall
  (~16.8M elems on the ACT engine, 1 elem/cycle/lane + ~0.5us fixed cost
  per instruction).  Everything is organized to keep ACT streaming wide
  back-to-back EXP instructions while the PE runs in long gap-free
  stretches (PE DVFS only reaches max clock after ~3us continuous busy).

  - PSUM: scores tiles [128,1536] x2 ping-pong (6 banks) + attnV quarter
    accumulators [65,512] x2 (2 banks) = all 8 banks.  exp instructions
    are 1536-wide: 22 per head instead of 32 -> ACT ~152us total.
  - exp SBUF: two banks (even/odd heads), each 2 half-tiles [128,16384]
    bf16 of flattened (t,s) columns.  The odd bank's second half aliases
    the phase-1 xt buffer (same pool tag -> byte reuse + auto WAR), which
    is what makes 2 x 64KB/partition fit; all xt readers are emitted
    before the first aliased exp write.
  - heads 0-2: attnV as t-outer quarter-pairs in the two 1-bank
    accumulators.  head 3 uses an s-major exp layout (quarter-blocks of
    512 s-cols x 16 t) so attnV(3) and the output projection pipeline
    per s-quarter behind exp(3) -> short tail.
  - output projection PSUM reuses the pv slots (the scores slots are
    still draining exp(3) then); copies/DMA rotate over vector/gpsimd/
    scalar/sync so ACT never issues DMA or copies mid-stream.
"""

import numpy as np
import ml_dtypes

BF16 = ml_dtypes.bfloat16

S = 2048  # sequence length
C = 1024  # d_model
NH = 16  # total heads
DK = 64  # head dim
N_CORES = 8
HPC = 4  # heads per core
DH = HPC * DK  # 256 per-core head dims
P = 128
VW = 72  # per-head stride in the V' buffer: 64 V cols + 1 ones col + 7 pad
HB = 16384  # exp half-bank width (8 t-tiles x 2048 s, or 2 q-blocks x 8192)

# exp tile pattern per 16384-col half-bank: 10 x 1536 + 1 x 1024
EXP_TILES = [(j * 1536, 1536) for j in range(10)] + [(15360, 1024)]
# head-3 (s-major) pattern per 8192-col quarter-block: 5 x 1536 + 1 x 512
EXP_TILES_Q = [(j * 1536, 1536) for j in range(5)] + [(7680, 512)]

_CACHE = {}


def _build_program():
    import concourse.bacc as bacc
    import concourse.mybir as mybir
    import concourse.tile as tile
    from contextlib import ExitStack

    dt = mybir.dt
    AF = mybir.ActivationFunctionType

    nc = bacc.Bacc("TRN2", target_bir_lowering=False, debug=False,
                   num_devices=N_CORES)

    xt = nc.dram_tensor("xt", [C, S], dt.bfloat16, kind="ExternalInput")
    wq = nc.dram_tensor("wq", [C, DH], dt.bfloat16, kind="ExternalInput")
    wk = nc.dram_tensor("wk", [C, DH], dt.bfloat16, kind="ExternalInput")
    wv = nc.dram_tensor("wv", [C, DH], dt.bfloat16, kind="ExternalInput")
    wo = nc.dram_tensor("wo", [DH, C], dt.bfloat16, kind="ExternalInput")
    # cols: [bq_tile0/8, bq_tile1/8, bk_tile0, bk_tile1]
    bqk = nc.dram_tensor("bqk", [P, 4], dt.float32, kind="ExternalInput")
    bv = nc.dram_tensor("bv", [1, DH], dt.bfloat16, kind="ExternalInput")
    outT = nc.dram_tensor("outT", [C, S], dt.bfloat16, kind="ExternalOutput")

    xt_r = xt.rearrange("(n p) s -> n p s", p=P)  # [8, 128, 2048]
    wq_r = wq.rearrange("(n p) d -> n p d", p=P)  # [8, 128, 256]
    wk_r = wk.rearrange("(n p) d -> n p d", p=P)
    wv_r = wv.rearrange("(n p) d -> n p d", p=P)
    wo_r = wo.rearrange("(n p) e -> n p e", p=P)  # [2, 128, 1024]
    outT_r = outT.rearrange("(n p) s -> n p s", p=P)  # [8, 128, 2048]

    with ExitStack() as ctx:
        tc = ctx.enter_context(tile.TileContext(nc))
        sb = ctx.enter_context(tc.tile_pool(name="sb", bufs=1))
        xpool = ctx.enter_context(tc.tile_pool(name="xpool", bufs=1))
        pool2 = ctx.enter_context(tc.tile_pool(name="pool2", bufs=1))
        spool = ctx.enter_context(tc.tile_pool(name="spool", bufs=2, space="PSUM"))
        vpool = ctx.enter_context(tc.tile_pool(name="vpool", bufs=2, space="PSUM"))

        # ---- persistent SBUF ----
        qt_sb = [sb.tile([P, S], dt.bfloat16, name=f"qt{i}", tag=f"qt{i}") for i in range(2)]
        kt_sb = [sb.tile([P, S], dt.bfloat16, name=f"kt{i}", tag=f"kt{i}") for i in range(2)]
        v_sb = [sb.tile([P, HPC * VW], dt.bfloat16, name=f"v{i}", tag=f"v{i}") for i in range(16)]
        hot_sb = [sb.tile([P, S], dt.bfloat16, name=f"ho{i}", tag=f"ho{i}") for i in range(2)]
        wo_sb = [sb.tile([P, C], dt.bfloat16, name=f"wo{i}", tag=f"wo{i}") for i in range(2)]
        bqk_sb = sb.tile([P, 4], dt.float32, name="bqk", tag="bqk")
        bv_sb = sb.tile([1, DH], dt.bfloat16, name="bv", tag="bv")
        ones_sb = sb.tile([1, P], dt.bfloat16, name="ones", tag="ones")
        # exp bank A (heads 0, 2): two fresh half-banks
        expA = [sb.tile([P, HB], dt.bfloat16, name=f"expA{i}", tag=f"expA{i}")
                for i in range(2)]
        # exp bank B (heads 1, 3): half 0 fresh; half 1 aliases xt later
        expB0 = sb.tile([P, HB], dt.bfloat16, name="expB0", tag="expB0")
        expB = [expB0, None]  # [1] filled after the last xt reader

        # ---- phase-1 SBUF (xpool) ----
        # xt as ONE tile: col block i*2048..(i+1)*2048 = c-chunk i of X^T.
        # Weights likewise single tiles with c-chunk i at cols i*DH.
        xt_sb = xpool.tile([P, 8 * S], dt.bfloat16, name="xbig", tag="xbig")
        wq_sb = xpool.tile([P, 8 * DH], dt.bfloat16, name="wqb", tag="wqb")
        wk_sb = xpool.tile([P, 8 * DH], dt.bfloat16, name="wkb", tag="wkb")
        wv_sb = xpool.tile([P, 8 * DH], dt.bfloat16, name="wvb", tag="wvb")

        # ---- loads ----
        # One batched 3D descriptor per xt s-quarter (sync queue, arrivals
        # ~3/6/9/12us) and per weight tensor (scalar queue, all by ~7us so
        # the first K/Q units start ~4us in; scalar is free again before
        # the first exp).
        xt_v = xt_sb.rearrange("p (n s) -> p n s", s=S)
        xt_p = xt.rearrange("(n p) s -> p n s", p=P)
        for q in range(4):
            nc.sync.dma_start(out=xt_v[:, :, q * 512:(q + 1) * 512],
                              in_=xt_p[:, :, q * 512:(q + 1) * 512])
        nc.scalar.dma_start(
            out=wk_sb.rearrange("p (n d) -> p n d", d=DH),
            in_=wk.rearrange("(n p) d -> p n d", p=P))
        nc.scalar.dma_start(
            out=wq_sb.rearrange("p (n d) -> p n d", d=DH),
            in_=wq.rearrange("(n p) d -> p n d", p=P))
        nc.scalar.dma_start(out=bqk_sb, in_=bqk[:, :])
        nc.scalar.dma_start(
            out=wv_sb.rearrange("p (n d) -> p n d", d=DH),
            in_=wv.rearrange("(n p) d -> p n d", p=P))
        nc.scalar.dma_start(out=bv_sb, in_=bv[:, :])
        nc.gpsimd.dma_start(out=wo_sb[0], in_=wo_r[0])
        nc.gpsimd.dma_start(out=wo_sb[1], in_=wo_r[1])
        nc.vector.memset(ones_sb, 1.0)
        for t in range(16):
            col = v_sb[t].rearrange("p (h w) -> p h w", w=VW)[:, :, DK:DK + 1]
            nc.vector.memset(col, 1.0)

        def exp_bank(h):
            if h % 2 == 0:
                return expA
            if expB[1] is None:
                # alias the xt bytes (same pool tag + size -> reuse + WAR)
                expB[1] = xpool.tile([P, HB], dt.bfloat16, name="expB1",
                                     tag="xbig")
            return expB

        def qkt_unit(qk, d2, sc):
            """One [128, 512] s-chunk of QT or KT for d-tile d2."""
            dst, w_sb, bias_col = (
                (qt_sb, wq_sb, 0) if qk == 0 else (kt_sb, wk_sb, 2)
            )
            ps = spool.tile([P, 512], dt.float32, name="mm", tag="mm")
            for c8 in range(8):
                nc.tensor.matmul(
                    ps,
                    lhsT=w_sb[:, c8 * DH + d2 * P: c8 * DH + (d2 + 1) * P],
                    rhs=xt_sb[:, c8 * S + sc * 512: c8 * S + (sc + 1) * 512],
                    start=(c8 == 0), stop=(c8 == 7),
                )
            nc.vector.tensor_scalar_add(
                dst[d2][:, sc * 512:(sc + 1) * 512], ps,
                bqk_sb[:, bias_col + d2:bias_col + d2 + 1],
            )

        def sc_tile(h, half, j):
            """Scores+exp tile j of half-bank `half` for head h (t-major:
            flattened col = (t%8)*2048 + s)."""
            c0, w = EXP_TILES[j]
            half_idx = h // 2
            row0 = (h % 2) * DK
            kth, qth = kt_sb[half_idx], qt_sb[half_idx]
            ps = spool.tile([P, w], dt.float32, name="mm", tag="mm")
            for cc in range(w // 512):
                c = c0 + cc * 512
                t = half * 8 + c // S
                s = c % S
                nc.tensor.matmul(
                    ps[:, cc * 512:(cc + 1) * 512],
                    lhsT=kth[row0:row0 + DK, t * P:(t + 1) * P],
                    rhs=qth[row0:row0 + DK, s:s + 512],
                    start=True, stop=True,
                )
            nc.scalar.activation(exp_bank(h)[half][:, c0:c0 + w], ps, AF.Exp)

        def sc3_tile(q, j):
            """Scores+exp tile for head 3, s-major: quarter-block q holds
            cols t*512 for all 16 t at s-quarter q."""
            c0, w = EXP_TILES_Q[j]
            row0 = DK  # head 3 = odd head of d-tile 1
            kth, qth = kt_sb[1], qt_sb[1]
            ps = spool.tile([P, w], dt.float32, name="mm", tag="mm")
            for cc in range(w // 512):
                t = (c0 + cc * 512) // 512
                nc.tensor.matmul(
                    ps[:, cc * 512:(cc + 1) * 512],
                    lhsT=kth[row0:row0 + DK, t * P:(t + 1) * P],
                    rhs=qth[row0:row0 + DK, q * 512:(q + 1) * 512],
                    start=True, stop=True,
                )
            dst = exp_bank(3)[q // 2]
            base = (q % 2) * 8192
            nc.scalar.activation(dst[:, base + c0: base + c0 + w], ps, AF.Exp)

        def vproj_t(t):
            # V tile t: [128, 256] + bias via ones-row; packed [64|1|pad] x4.
            ps = spool.tile([P, DH], dt.float32, name="mm", tag="mm")
            for c8 in range(8):
                nc.tensor.matmul(
                    ps, lhsT=xt_sb[:, c8 * S + t * P: c8 * S + (t + 1) * P],
                    rhs=wv_sb[:, c8 * DH:(c8 + 1) * DH],
                    start=(c8 == 0), stop=False,
                )
            nc.tensor.matmul(ps, lhsT=ones_sb, rhs=bv_sb,
                             start=False, stop=True)
            dst = v_sb[t].rearrange("p (h w) -> p h w", w=VW)[:, :, 0:DK]
            src = ps.rearrange("p (h w) -> p h w", w=DK)
            nc.vector.tensor_copy(dst, src)

        def norm_q(h, q, pv):
            """Normalize attnV s-quarter q of head h out of accumulator pv."""
            half_idx = h // 2
            row0 = (h % 2) * DK
            cs = slice(q * 512, (q + 1) * 512)
            sumtmp = pool2.tile([1, 512], dt.float32, name="sumtmp",
                                tag="sumtmp", bufs=2)
            sr = pool2.tile([1, 512], dt.float32, name="sr", tag="sr", bufs=2)
            rbc = pool2.tile([DK, 512], dt.float32, name="rbc", tag="rbc",
                             bufs=2)
            nc.vector.tensor_copy(sumtmp, pv[DK:DK + 1, :])
            nc.vector.reciprocal_approx_fast(sr, sumtmp)
            nc.gpsimd.partition_broadcast(rbc, sr)
            nc.vector.tensor_mul(
                hot_sb[half_idx][row0:row0 + DK, cs], pv[:DK, :], rbc,
            )

        def attnv_pair(h, pair):
            """attnV s-quarters (2*pair, 2*pair+1) of head h, t-outer over
            two 1-bank accumulators (t-major exp layout, heads 0-2)."""
            eh = exp_bank(h)
            q0, q1 = 2 * pair, 2 * pair + 1
            pva = vpool.tile([DK + 1, 512], dt.float32, name="pv", tag="pv")
            pvb = vpool.tile([DK + 1, 512], dt.float32, name="pv", tag="pv")
            for t in range(16):
                half = t // 8
                base = (t % 8) * S
                for q, pv in ((q0, pva), (q1, pvb)):
                    nc.tensor.matmul(
                        pv,
                        lhsT=v_sb[t][:, h * VW:h * VW + DK + 1],
                        rhs=eh[half][:, base + q * 512: base + (q + 1) * 512],
                        start=(t == 0), stop=(t == 15),
                    )
            norm_q(h, q0, pva)
            norm_q(h, q1, pvb)

        def attnv3_q(q):
            """attnV s-quarter q of head 3 (s-major layout): rides exp(3)."""
            eh = exp_bank(3)[q // 2]
            base = (q % 2) * 8192
            pv = vpool.tile([DK + 1, 512], dt.float32, name="pv", tag="pv")
            for t in range(16):
                nc.tensor.matmul(
                    pv,
                    lhsT=v_sb[t][:, 3 * VW:3 * VW + DK + 1],
                    rhs=eh[:, base + t * 512: base + (t + 1) * 512],
                    start=(t == 0), stop=(t == 15),
                )
            norm_q(3, q, pv)

        def outproj_sq(q):
            """Output projection for s-quarter q. PSUM from the pv slots
            (scores slots are still draining exp(3)); copies and DMA
            rotate engines; ACT is free at the tail so it helps too."""
            for e in range(8):
                ps = vpool.tile([P, 512], dt.float32, name="pv", tag="pv")
                for d2 in range(2):
                    nc.tensor.matmul(
                        ps,
                        lhsT=wo_sb[d2][:, e * P:(e + 1) * P],
                        rhs=hot_sb[d2][:, q * 512:(q + 1) * 512],
                        start=(d2 == 0), stop=(d2 == 1),
                    )
                st = pool2.tile([P, 512], dt.bfloat16, name="st",
                                tag="st", bufs=3)
                if e % 2 == 0:
                    nc.vector.tensor_copy(st, ps)
                else:
                    nc.scalar.copy(st, ps)
                (nc.sync, nc.gpsimd)[e % 2].dma_start(
                    out=outT_r[e][:, q * 512:(q + 1) * 512], in_=st)

        # ================= emission =================
        # Phase A: QKT d2=0 fine-grained, interleaved with head-0 score
        # tiles so ACT starts exp(0) ~10us in.
        qkt_unit(1, 0, 0)               # K d2=0 s 0:512
        qkt_unit(0, 0, 0)               # Q s 0:512
        qkt_unit(0, 0, 1)
        qkt_unit(0, 0, 2)
        sc_tile(0, 0, 0)
        qkt_unit(0, 0, 3)
        sc_tile(0, 0, 1)
        qkt_unit(1, 0, 1)
        sc_tile(0, 0, 2)
        sc_tile(0, 0, 3)
        qkt_unit(1, 0, 2)
        sc_tile(0, 0, 4)
        sc_tile(0, 0, 5)
        qkt_unit(1, 0, 3)
        for j in range(6, 11):
            sc_tile(0, 0, j)
        for j in range(11):
            sc_tile(0, 1, j)
        # head-1 scores (fresh half) paced against vproj + QKT d2=1 so the
        # aliased half-1 tiles come after every xt reader.
        for j in range(4):
            sc_tile(1, 0, j)
        for t in range(8):
            vproj_t(t)
        for j in range(4, 7):
            sc_tile(1, 0, j)
        for t in range(8, 16):
            vproj_t(t)
        for j in range(7, 9):
            sc_tile(1, 0, j)
        qkt_unit(1, 1, 0)
        qkt_unit(1, 1, 1)
        qkt_unit(0, 1, 0)
        qkt_unit(0, 1, 1)
        for j in range(9, 11):
            sc_tile(1, 0, j)
        qkt_unit(1, 1, 2)
        qkt_unit(1, 1, 3)
        qkt_unit(0, 1, 2)
        qkt_unit(0, 1, 3)
        # all xt readers done -> aliased exp half is safe from here
        for j in range(11):
            sc_tile(1, 1, j)
        attnv_pair(0, 0)
        attnv_pair(0, 1)
        # scores head 2 feed exp(2) (bank A, freed by attnv(0))
        for half in range(2):
            for j in range(11):
                sc_tile(2, half, j)
        attnv_pair(1, 0)
        attnv_pair(1, 1)
        # scores head 3, s-major quarter-blocks
        for q in range(4):
            for j in range(6):
                sc3_tile(q, j)
        attnv_pair(2, 0)
        attnv_pair(2, 1)
        # tail: head-3 attnV + output projection pipeline per s-quarter
        for q in range(4):
            attnv3_q(q)
            outproj_sq(q)

    nc.compile()
    return nc


def _get_program():
    if "nc" not in _CACHE:
        _CACHE["nc"] = _build_program()
    return _CACHE["nc"]


def _shard_inputs(input, W_qkv, b_qkv, W_out):
    """Build the 8 per-core input maps (host-side shard + transpose + cast)."""
    in_maps = []
    xt_by_b = [
        np.ascontiguousarray(input[b].T).astype(BF16) for b in range(2)
    ]
    for core in range(N_CORES):
        b, g = divmod(core, HPC)
        cols = slice(g * DH, (g + 1) * DH)
        bq = (b_qkv[g * DH:(g + 1) * DH] / 8.0).astype(np.float32)
        bk = b_qkv[C + g * DH:C + (g + 1) * DH].astype(np.float32)
        bqk = np.stack([bq[:P], bq[P:], bk[:P], bk[P:]], axis=1)
        in_maps.append({
            "xt": xt_by_b[b],
            "wq": np.ascontiguousarray(W_qkv[:, cols] * 0.125).astype(BF16),
            "wk": np.ascontiguousarray(W_qkv[:, C:2 * C][:, cols]).astype(BF16),
            "wv": np.ascontiguousarray(W_qkv[:, 2 * C:][:, cols]).astype(BF16),
            "wo": np.ascontiguousarray(W_out[g * DH:(g + 1) * DH, :]).astype(BF16),
            "bqk": np.ascontiguousarray(bqk, dtype=np.float32),
            "bv": b_qkv[2 * C + g * DH:2 * C + (g + 1) * DH]
                  .astype(BF16).reshape(1, DH),
        })
    return in_maps


def kernel(input, W_qkv, b_qkv, W_out):
    from concourse.bass_utils import run_bass_kernel_spmd

    nc = _get_program()
    in_maps = _shard_inputs(
        np.asarray(input), np.asarray(W_qkv), np.asarray(b_qkv),
        np.asarray(W_out),
    )
    res = run_bass_kernel_spmd(nc, in_maps, core_ids=list(range(N_CORES)))
    out = np.zeros((2, S, C), dtype=np.float32)
    for core in range(N_CORES):
        b = core // HPC
        out[b] += np.asarray(res.results[core]["outT"]).astype(np.float32).T
    return out


# revision 14
# speedup vs baseline: 1.1130x; 1.0844x over previous
"""Multi-head attention (B=2, S=2048, D=1024, H=16) on 8 Trainium2 cores.

Sharding: 2-way data parallel on batch x 4-way tensor parallel on heads.
Core c handles batch b = c // 4 and head group g = c % 4 (4 heads, 256 dims).

v2 design (ACT-bound pipeline):
  The exp over the per-head [S,S] score matrices is the per-core wall
  (~16.8M elems on the ACT engine, 1 elem/cycle/lane + ~0.5us fixed cost
  per instruction).  Everything is organized to keep ACT streaming wide
  back-to-back EXP instructions while the PE runs in long gap-free
  stretches (PE DVFS only reaches max clock after ~3us continuous busy).

  - PSUM: scores tiles [128,1536] x2 ping-pong (6 banks) + attnV quarter
    accumulators [65,512] x2 (2 banks) = all 8 banks.  exp instructions
    are 1536-wide: 22 per head instead of 32 -> ACT ~152us total.
  - exp SBUF: two banks (even/odd heads), each 2 half-tiles [128,16384]
    bf16 of flattened (t,s) columns.  The odd bank's second half aliases
    the phase-1 xt buffer (same pool tag -> byte reuse + auto WAR), which
    is what makes 2 x 64KB/partition fit; all xt readers are emitted
    before the first aliased exp write.
  - heads 0-2: attnV as t-outer quarter-pairs in the two 1-bank
    accumulators.  head 3 uses an s-major exp layout (quarter-blocks of
    512 s-cols x 16 t) so attnV(3) and the output projection pipeline
    per s-quarter behind exp(3) -> short tail.
  - output projection PSUM reuses the pv slots (the scores slots are
    still draining exp(3) then); copies/DMA rotate over vector/gpsimd/
    scalar/sync so ACT never issues DMA or copies mid-stream.
"""

import numpy as np
import ml_dtypes

BF16 = ml_dtypes.bfloat16

S = 2048  # sequence length
C = 1024  # d_model
NH = 16  # total heads
DK = 64  # head dim
N_CORES = 8
HPC = 4  # heads per core
DH = HPC * DK  # 256 per-core head dims
P = 128
VW = 72  # per-head stride in the V' buffer: 64 V cols + 1 ones col + 7 pad
HB = 16384  # exp half-bank width (8 t-tiles x 2048 s, or 2 q-blocks x 8192)

# exp tile pattern per 16384-col half-bank: 10 x 1536 + 1 x 1024
EXP_TILES = [(j * 1536, 1536) for j in range(10)] + [(15360, 1024)]
# head-3 (s-major) pattern per 8192-col quarter-block: 5 x 1536 + 1 x 512
EXP_TILES_Q = [(j * 1536, 1536) for j in range(5)] + [(7680, 512)]

_CACHE = {}


def _build_program():
    import concourse.bacc as bacc
    import concourse.mybir as mybir
    import concourse.tile as tile
    from contextlib import ExitStack

    dt = mybir.dt
    AF = mybir.ActivationFunctionType

    nc = bacc.Bacc("TRN2", target_bir_lowering=False, debug=False,
                   num_devices=N_CORES)

    xt = nc.dram_tensor("xt", [C, S], dt.bfloat16, kind="ExternalInput")
    wq = nc.dram_tensor("wq", [C, DH], dt.bfloat16, kind="ExternalInput")
    wk = nc.dram_tensor("wk", [C, DH], dt.bfloat16, kind="ExternalInput")
    wv = nc.dram_tensor("wv", [C, DH], dt.bfloat16, kind="ExternalInput")
    wo = nc.dram_tensor("wo", [DH, C], dt.bfloat16, kind="ExternalInput")
    # cols: [bq_tile0/8, bq_tile1/8, bk_tile0, bk_tile1]
    bqk = nc.dram_tensor("bqk", [P, 4], dt.float32, kind="ExternalInput")
    bv = nc.dram_tensor("bv", [1, DH], dt.bfloat16, kind="ExternalInput")
    outT = nc.dram_tensor("outT", [C, S], dt.bfloat16, kind="ExternalOutput")

    xt_r = xt.rearrange("(n p) s -> n p s", p=P)  # [8, 128, 2048]
    wq_r = wq.rearrange("(n p) d -> n p d", p=P)  # [8, 128, 256]
    wk_r = wk.rearrange("(n p) d -> n p d", p=P)
    wv_r = wv.rearrange("(n p) d -> n p d", p=P)
    wo_r = wo.rearrange("(n p) e -> n p e", p=P)  # [2, 128, 1024]
    outT_r = outT.rearrange("(n p) s -> n p s", p=P)  # [8, 128, 2048]

    with ExitStack() as ctx:
        tc = ctx.enter_context(tile.TileContext(nc))
        sb = ctx.enter_context(tc.tile_pool(name="sb", bufs=1))
        xpool = ctx.enter_context(tc.tile_pool(name="xpool", bufs=1))
        pool2 = ctx.enter_context(tc.tile_pool(name="pool2", bufs=1))
        spool = ctx.enter_context(tc.tile_pool(name="spool", bufs=2, space="PSUM"))
        vpool = ctx.enter_context(tc.tile_pool(name="vpool", bufs=2, space="PSUM"))

        # ---- persistent SBUF ----
        qt_sb = [sb.tile([P, S], dt.bfloat16, name=f"qt{i}", tag=f"qt{i}") for i in range(2)]
        kt_sb = [sb.tile([P, S], dt.bfloat16, name=f"kt{i}", tag=f"kt{i}") for i in range(2)]
        v_sb = [sb.tile([P, HPC * VW], dt.bfloat16, name=f"v{i}", tag=f"v{i}") for i in range(16)]
        hot_sb = [sb.tile([P, S], dt.bfloat16, name=f"ho{i}", tag=f"ho{i}") for i in range(2)]
        wo_sb = [sb.tile([P, C], dt.bfloat16, name=f"wo{i}", tag=f"wo{i}") for i in range(2)]
        bqk_sb = sb.tile([P, 4], dt.float32, name="bqk", tag="bqk")
        bv_sb = sb.tile([1, DH], dt.bfloat16, name="bv", tag="bv")
        ones_sb = sb.tile([1, P], dt.bfloat16, name="ones", tag="ones")
        # exp bank A (heads 0, 2): two fresh half-banks
        expA = [sb.tile([P, HB], dt.bfloat16, name=f"expA{i}", tag=f"expA{i}")
                for i in range(2)]
        # exp bank B (heads 1, 3): half 0 fresh; half 1 aliases xt later
        expB0 = sb.tile([P, HB], dt.bfloat16, name="expB0", tag="expB0")
        expB = [expB0, None]  # [1] filled after the last xt reader

        # ---- phase-1 SBUF (xpool) ----
        # xt as ONE tile: col block i*2048..(i+1)*2048 = c-chunk i of X^T.
        # Weights likewise single tiles with c-chunk i at cols i*DH.
        xt_sb = xpool.tile([P, 8 * S], dt.bfloat16, name="xbig", tag="xbig")
        wq_sb = xpool.tile([P, 8 * DH], dt.bfloat16, name="wqb", tag="wqb")
        wk_sb = xpool.tile([P, 8 * DH], dt.bfloat16, name="wkb", tag="wkb")
        wv_sb = xpool.tile([P, 8 * DH], dt.bfloat16, name="wvb", tag="wvb")

        # ---- loads ----
        # One batched 3D descriptor per xt s-quarter (sync queue, arrivals
        # ~3/6/9/12us) and per weight tensor (scalar queue, all by ~7us so
        # the first K/Q units start ~4us in; scalar is free again before
        # the first exp).
        xt_v = xt_sb.rearrange("p (n s) -> p n s", s=S)
        xt_p = xt.rearrange("(n p) s -> p n s", p=P)

        def xt_quarter(q):
            return (xt_v[:, :, q * 512:(q + 1) * 512],
                    xt_p[:, :, q * 512:(q + 1) * 512])

        # sync: q0, q2; scalar: wk, wq, q1, q3, biases, wv — so K00/Q00 can
        # start ~5us in and quarter q arrives before its first consumer.
        o, i = xt_quarter(0)
        nc.sync.dma_start(out=o, in_=i)
        o, i = xt_quarter(2)
        nc.sync.dma_start(out=o, in_=i)
        nc.scalar.dma_start(
            out=wk_sb.rearrange("p (n d) -> p n d", d=DH),
            in_=wk.rearrange("(n p) d -> p n d", p=P))
        nc.scalar.dma_start(
            out=wq_sb.rearrange("p (n d) -> p n d", d=DH),
            in_=wq.rearrange("(n p) d -> p n d", p=P))
        o, i = xt_quarter(1)
        nc.scalar.dma_start(out=o, in_=i)
        o, i = xt_quarter(3)
        nc.scalar.dma_start(out=o, in_=i)
        nc.scalar.dma_start(out=bqk_sb, in_=bqk[:, :])
        nc.scalar.dma_start(
            out=wv_sb.rearrange("p (n d) -> p n d", d=DH),
            in_=wv.rearrange("(n p) d -> p n d", p=P))
        nc.scalar.dma_start(out=bv_sb, in_=bv[:, :])
        nc.gpsimd.dma_start(out=wo_sb[0], in_=wo_r[0])
        nc.gpsimd.dma_start(out=wo_sb[1], in_=wo_r[1])
        nc.vector.memset(ones_sb, 1.0)
        for t in range(16):
            col = v_sb[t].rearrange("p (h w) -> p h w", w=VW)[:, :, DK:DK + 1]
            nc.vector.memset(col, 1.0)

        def exp_bank(h):
            if h % 2 == 0:
                return expA
            if expB[1] is None:
                # alias the xt bytes (same pool tag + size -> reuse + WAR)
                expB[1] = xpool.tile([P, HB], dt.bfloat16, name="expB1",
                                     tag="xbig")
            return expB

        def qkt_unit(qk, d2, sc):
            """One [128, 512] s-chunk of QT or KT for d-tile d2."""
            dst, w_sb, bias_col = (
                (qt_sb, wq_sb, 0) if qk == 0 else (kt_sb, wk_sb, 2)
            )
            ps = spool.tile([P, 512], dt.float32, name="mm", tag="mm")
            for c8 in range(8):
                nc.tensor.matmul(
                    ps,
                    lhsT=w_sb[:, c8 * DH + d2 * P: c8 * DH + (d2 + 1) * P],
                    rhs=xt_sb[:, c8 * S + sc * 512: c8 * S + (sc + 1) * 512],
                    start=(c8 == 0), stop=(c8 == 7),
                )
            nc.vector.tensor_scalar_add(
                dst[d2][:, sc * 512:(sc + 1) * 512], ps,
                bqk_sb[:, bias_col + d2:bias_col + d2 + 1],
            )

        def sc_tile(h, half, j):
            """Scores+exp tile j of half-bank `half` for head h (t-major:
            flattened col = (t%8)*2048 + s)."""
            c0, w = EXP_TILES[j]
            half_idx = h // 2
            row0 = (h % 2) * DK
            kth, qth = kt_sb[half_idx], qt_sb[half_idx]
            ps = spool.tile([P, w], dt.float32, name="mm", tag="mm")
            for cc in range(w // 512):
                c = c0 + cc * 512
                t = half * 8 + c // S
                s = c % S
                nc.tensor.matmul(
                    ps[:, cc * 512:(cc + 1) * 512],
                    lhsT=kth[row0:row0 + DK, t * P:(t + 1) * P],
                    rhs=qth[row0:row0 + DK, s:s + 512],
                    start=True, stop=True,
                )
            nc.scalar.activation(exp_bank(h)[half][:, c0:c0 + w], ps, AF.Exp)

        def sc3_tile(q, j):
            """Scores+exp tile for head 3, s-major: quarter-block q holds
            cols t*512 for all 16 t at s-quarter q."""
            c0, w = EXP_TILES_Q[j]
            row0 = DK  # head 3 = odd head of d-tile 1
            kth, qth = kt_sb[1], qt_sb[1]
            ps = spool.tile([P, w], dt.float32, name="mm", tag="mm")
            for cc in range(w // 512):
                t = (c0 + cc * 512) // 512
                nc.tensor.matmul(
                    ps[:, cc * 512:(cc + 1) * 512],
                    lhsT=kth[row0:row0 + DK, t * P:(t + 1) * P],
                    rhs=qth[row0:row0 + DK, q * 512:(q + 1) * 512],
                    start=True, stop=True,
                )
            dst = exp_bank(3)[q // 2]
            base = (q % 2) * 8192
            nc.scalar.activation(dst[:, base + c0: base + c0 + w], ps, AF.Exp)

        def vproj_t(t):
            # V tile t: [128, 256] + bias via ones-row; packed [64|1|pad] x4.
            ps = spool.tile([P, DH], dt.float32, name="mm", tag="mm")
            for c8 in range(8):
                nc.tensor.matmul(
                    ps, lhsT=xt_sb[:, c8 * S + t * P: c8 * S + (t + 1) * P],
                    rhs=wv_sb[:, c8 * DH:(c8 + 1) * DH],
                    start=(c8 == 0), stop=False,
                )
            nc.tensor.matmul(ps, lhsT=ones_sb, rhs=bv_sb,
                             start=False, stop=True)
            dst = v_sb[t].rearrange("p (h w) -> p h w", w=VW)[:, :, 0:DK]
            src = ps.rearrange("p (h w) -> p h w", w=DK)
            nc.vector.tensor_copy(dst, src)

        def norm_q(h, q, pv):
            """Normalize attnV s-quarter q of head h out of accumulator pv."""
            half_idx = h // 2
            row0 = (h % 2) * DK
            cs = slice(q * 512, (q + 1) * 512)
            sumtmp = pool2.tile([1, 512], dt.float32, name="sumtmp",
                                tag="sumtmp", bufs=2)
            sr = pool2.tile([1, 512], dt.float32, name="sr", tag="sr", bufs=2)
            rbc = pool2.tile([DK, 512], dt.float32, name="rbc", tag="rbc",
                             bufs=2)
            nc.vector.tensor_copy(sumtmp, pv[DK:DK + 1, :])
            nc.vector.reciprocal_approx_fast(sr, sumtmp)
            nc.gpsimd.partition_broadcast(rbc, sr)
            nc.vector.tensor_mul(
                hot_sb[half_idx][row0:row0 + DK, cs], pv[:DK, :], rbc,
            )

        def attnv_pair_begin(h, pair):
            """attnV s-quarters (2*pair, 2*pair+1) of head h, t-outer over
            two 1-bank accumulators (t-major exp layout, heads 0-2)."""
            pva = vpool.tile([DK + 1, 512], dt.float32, name="pv", tag="pv")
            pvb = vpool.tile([DK + 1, 512], dt.float32, name="pv", tag="pv")
            return (h, pair, pva, pvb)

        def attnv_pair_step(actx, t):
            h, pair, pva, pvb = actx
            eh = exp_bank(h)
            half = t // 8
            base = (t % 8) * S
            for q, pv in ((2 * pair, pva), (2 * pair + 1, pvb)):
                nc.tensor.matmul(
                    pv,
                    lhsT=v_sb[t][:, h * VW:h * VW + DK + 1],
                    rhs=eh[half][:, base + q * 512: base + (q + 1) * 512],
                    start=(t == 0), stop=(t == 15),
                )

        def attnv_pair_end(actx):
            h, pair, pva, pvb = actx
            norm_q(h, 2 * pair, pva)
            norm_q(h, 2 * pair + 1, pvb)

        def attnv3_q(q):
            """attnV s-quarter q of head 3 (s-major layout): rides exp(3)."""
            eh = exp_bank(3)[q // 2]
            base = (q % 2) * 8192
            pv = vpool.tile([DK + 1, 512], dt.float32, name="pv", tag="pv")
            for t in range(16):
                nc.tensor.matmul(
                    pv,
                    lhsT=v_sb[t][:, 3 * VW:3 * VW + DK + 1],
                    rhs=eh[:, base + t * 512: base + (t + 1) * 512],
                    start=(t == 0), stop=(t == 15),
                )
            norm_q(3, q, pv)

        def outproj_sq(q):
            """Output projection for s-quarter q. PSUM from the pv slots
            (scores slots are still draining exp(3)); copies and DMA
            rotate engines; ACT is free at the tail so it helps too."""
            for e in range(8):
                ps = vpool.tile([P, 512], dt.float32, name="pv", tag="pv")
                for d2 in range(2):
                    nc.tensor.matmul(
                        ps,
                        lhsT=wo_sb[d2][:, e * P:(e + 1) * P],
                        rhs=hot_sb[d2][:, q * 512:(q + 1) * 512],
                        start=(d2 == 0), stop=(d2 == 1),
                    )
                st = pool2.tile([P, 512], dt.bfloat16, name="st",
                                tag="st", bufs=3)
                if e % 2 == 0:
                    nc.vector.tensor_copy(st, ps)
                else:
                    nc.scalar.copy(st, ps)
                (nc.sync, nc.gpsimd)[e % 2].dma_start(
                    out=outT_r[e][:, q * 512:(q + 1) * 512], in_=st)

        # ================= emission =================
        # Four balanced streams: stream h runs head h's score tiles (the
        # food for exp(h) on ACT) interleaved ~1:1 with filler PE work
        # whose deps are already satisfied, so neither engine ever idles
        # long: S0 filler = QKT d2=0 + vproj, S1 = attnV(0) + QKT d2=1,
        # S2 = attnV(1), S3 = attnV(2).  Tail = attnV(3) riding exp(3)
        # s-major + the output projection per s-quarter.

        def interleave(tiles, fillers):
            """Emit score tiles and filler thunks, pacing tiles evenly
            through the filler list."""
            nt, nf = len(tiles), len(fillers)
            ti = fi = 0
            while ti < nt or fi < nf:
                # keep tile emission slightly ahead of even pacing
                if ti < nt and (fi >= nf or ti * nf <= fi * nt):
                    tiles[ti]()
                    ti += 1
                else:
                    fillers[fi]()
                    fi += 1

        def T(h, half, j):
            return lambda: sc_tile(h, half, j)

        def T3(q, j):
            return lambda: sc3_tile(q, j)

        def attn_steps(actx, ts):
            return [lambda t=t: attnv_pair_step(actx, t) for t in ts]

        # ---- stream 0 ----
        # hand-ordered prologue so every score tile follows its K/Q units
        # (PE executes in issue order: a tile emitted before its unit would
        # head-block the PE stream forever).
        qkt_unit(1, 0, 0)               # K d2=0 s 0:512
        qkt_unit(0, 0, 0)               # Q s 0:512
        qkt_unit(0, 0, 1)
        qkt_unit(0, 0, 2)
        sc_tile(0, 0, 0)
        qkt_unit(0, 0, 3)
        sc_tile(0, 0, 1)
        qkt_unit(1, 0, 1)
        sc_tile(0, 0, 2)
        sc_tile(0, 0, 3)
        qkt_unit(1, 0, 2)
        sc_tile(0, 0, 4)
        sc_tile(0, 0, 5)
        qkt_unit(1, 0, 3)
        s0_tiles = ([T(0, 0, j) for j in range(6, 11)]
                    + [T(0, 1, j) for j in range(11)])
        interleave(s0_tiles, [lambda t=t: vproj_t(t) for t in range(16)])

        # ---- stream 1 ----
        # fresh half first (no xt dep) against attnV(0) pair 0 + QKT d2=1;
        # the aliased half follows once every xt reader is emitted.
        a0p0 = attnv_pair_begin(0, 0)
        s1_fill_a = (attn_steps(a0p0, range(8))
                     + [lambda qk=qk, sc=sc: qkt_unit(qk, 1, sc)
                        for qk in (1, 0) for sc in range(4)])
        interleave([T(1, 0, j) for j in range(11)], s1_fill_a)
        s1_post = [T(1, 1, j) for j in range(11)]
        interleave(s1_post[:5], attn_steps(a0p0, range(8, 16)))
        attnv_pair_end(a0p0)
        a0p1 = attnv_pair_begin(0, 1)
        interleave(s1_post[5:], attn_steps(a0p1, range(16)))
        attnv_pair_end(a0p1)

        # ---- streams 2 and 3 ----
        for h, tiles in (
            (1, [T(2, half, j) for half in range(2) for j in range(11)]),
            (2, [T3(q, j) for q in range(4) for j in range(6)]),
        ):
            ap0 = attnv_pair_begin(h, 0)
            interleave(tiles[:len(tiles) // 2], attn_steps(ap0, range(16)))
            attnv_pair_end(ap0)
            ap1 = attnv_pair_begin(h, 1)
            interleave(tiles[len(tiles) // 2:], attn_steps(ap1, range(16)))
            attnv_pair_end(ap1)

        # ---- tail: head-3 attnV rides exp(3); outproj per s-quarter ----
        for q in range(4):
            attnv3_q(q)
            outproj_sq(q)

    nc.compile()
    return nc


def _get_program():
    if "nc" not in _CACHE:
        _CACHE["nc"] = _build_program()
    return _CACHE["nc"]


def _shard_inputs(input, W_qkv, b_qkv, W_out):
    """Build the 8 per-core input maps (host-side shard + transpose + cast)."""
    in_maps = []
    xt_by_b = [
        np.ascontiguousarray(input[b].T).astype(BF16) for b in range(2)
    ]
    for core in range(N_CORES):
        b, g = divmod(core, HPC)
        cols = slice(g * DH, (g + 1) * DH)
        bq = (b_qkv[g * DH:(g + 1) * DH] / 8.0).astype(np.float32)
        bk = b_qkv[C + g * DH:C + (g + 1) * DH].astype(np.float32)
        bqk = np.stack([bq[:P], bq[P:], bk[:P], bk[P:]], axis=1)
        in_maps.append({
            "xt": xt_by_b[b],
            "wq": np.ascontiguousarray(W_qkv[:, cols] * 0.125).astype(BF16),
            "wk": np.ascontiguousarray(W_qkv[:, C:2 * C][:, cols]).astype(BF16),
            "wv": np.ascontiguousarray(W_qkv[:, 2 * C:][:, cols]).astype(BF16),
            "wo": np.ascontiguousarray(W_out[g * DH:(g + 1) * DH, :]).astype(BF16),
            "bqk": np.ascontiguousarray(bqk, dtype=np.float32),
            "bv": b_qkv[2 * C + g * DH:2 * C + (g + 1) * DH]
                  .astype(BF16).reshape(1, DH),
        })
    return in_maps


def kernel(input, W_qkv, b_qkv, W_out):
    from concourse.bass_utils import run_bass_kernel_spmd

    nc = _get_program()
    in_maps = _shard_inputs(
        np.asarray(input), np.asarray(W_qkv), np.asarray(b_qkv),
        np.asarray(W_out),
    )
    res = run_bass_kernel_spmd(nc, in_maps, core_ids=list(range(N_CORES)))
    out = np.zeros((2, S, C), dtype=np.float32)
    for core in range(N_CORES):
        b = core // HPC
        out[b] += np.asarray(res.results[core]["outT"]).astype(np.float32).T
    return out


# revision 20
# speedup vs baseline: 1.1906x; 1.0697x over previous
"""Multi-head attention (B=2, S=2048, D=1024, H=16) on 8 Trainium2 cores.

Sharding: 2-way data parallel on batch x 4-way tensor parallel on heads.
Core c handles batch b = c // 4 and head group g = c % 4 (4 heads, 256 dims).

v2 design (ACT-bound pipeline):
  The exp over the per-head [S,S] score matrices is the per-core wall
  (~16.8M elems on the ACT engine, 1 elem/cycle/lane + ~0.5us fixed cost
  per instruction).  Everything is organized to keep ACT streaming wide
  back-to-back EXP instructions while the PE runs in long gap-free
  stretches (PE DVFS only reaches max clock after ~3us continuous busy).

  - PSUM: scores tiles [128,1536] x2 ping-pong (6 banks) + attnV quarter
    accumulators [65,512] x2 (2 banks) = all 8 banks.  exp instructions
    are 1536-wide: 22 per head instead of 32 -> ACT ~152us total.
  - exp SBUF: two banks (even/odd heads), each 2 half-tiles [128,16384]
    bf16 of flattened (t,s) columns.  The odd bank's second half aliases
    the phase-1 xt buffer (same pool tag -> byte reuse + auto WAR), which
    is what makes 2 x 64KB/partition fit; all xt readers are emitted
    before the first aliased exp write.
  - heads 0-2: attnV as t-outer quarter-pairs in the two 1-bank
    accumulators.  head 3 uses an s-major exp layout (quarter-blocks of
    512 s-cols x 16 t) so attnV(3) and the output projection pipeline
    per s-quarter behind exp(3) -> short tail.
  - output projection PSUM reuses the pv slots (the scores slots are
    still draining exp(3) then); copies/DMA rotate over vector/gpsimd/
    scalar/sync so ACT never issues DMA or copies mid-stream.
"""

import numpy as np
import ml_dtypes

BF16 = ml_dtypes.bfloat16

S = 2048  # sequence length
C = 1024  # d_model
NH = 16  # total heads
DK = 64  # head dim
N_CORES = 8
HPC = 4  # heads per core
DH = HPC * DK  # 256 per-core head dims
P = 128
VW = 72  # per-head stride in the V' buffer: 64 V cols + 1 ones col + 7 pad
HB = 16384  # exp half-bank width (8 t-tiles x 2048 s, or 2 q-blocks x 8192)

# exp tile pattern per 16384-col half-bank: 10 x 1536 + 1 x 1024
EXP_TILES = [(j * 1536, 1536) for j in range(10)] + [(15360, 1024)]
# head-3 (s-major) pattern per 8192-col quarter-block: 5 x 1536 + 1 x 512
EXP_TILES_Q = [(j * 1536, 1536) for j in range(5)] + [(7680, 512)]

_CACHE = {}


def _build_program():
    import concourse.bacc as bacc
    import concourse.mybir as mybir
    import concourse.tile as tile
    from contextlib import ExitStack

    dt = mybir.dt
    AF = mybir.ActivationFunctionType

    nc = bacc.Bacc("TRN2", target_bir_lowering=False, debug=False,
                   num_devices=N_CORES)

    xt = nc.dram_tensor("xt", [C, S], dt.bfloat16, kind="ExternalInput")
    wq = nc.dram_tensor("wq", [C, DH], dt.bfloat16, kind="ExternalInput")
    wk = nc.dram_tensor("wk", [C, DH], dt.bfloat16, kind="ExternalInput")
    wv = nc.dram_tensor("wv", [C, DH], dt.bfloat16, kind="ExternalInput")
    wo = nc.dram_tensor("wo", [DH, C], dt.bfloat16, kind="ExternalInput")
    # cols: [bq_tile0/8, bq_tile1/8, bk_tile0, bk_tile1]
    bqk = nc.dram_tensor("bqk", [P, 4], dt.float32, kind="ExternalInput")
    bv = nc.dram_tensor("bv", [1, DH], dt.bfloat16, kind="ExternalInput")
    outT = nc.dram_tensor("outT", [C, S], dt.bfloat16, kind="ExternalOutput")

    xt_r = xt.rearrange("(n p) s -> n p s", p=P)  # [8, 128, 2048]
    wq_r = wq.rearrange("(n p) d -> n p d", p=P)  # [8, 128, 256]
    wk_r = wk.rearrange("(n p) d -> n p d", p=P)
    wv_r = wv.rearrange("(n p) d -> n p d", p=P)
    wo_r = wo.rearrange("(n p) e -> n p e", p=P)  # [2, 128, 1024]
    outT_r = outT.rearrange("(n p) s -> n p s", p=P)  # [8, 128, 2048]

    with ExitStack() as ctx:
        tc = ctx.enter_context(tile.TileContext(nc))
        sb = ctx.enter_context(tc.tile_pool(name="sb", bufs=1))
        xpool = ctx.enter_context(tc.tile_pool(name="xpool", bufs=1))
        pool2 = ctx.enter_context(tc.tile_pool(name="pool2", bufs=1))
        spool = ctx.enter_context(tc.tile_pool(name="spool", bufs=2, space="PSUM"))
        vpool = ctx.enter_context(tc.tile_pool(name="vpool", bufs=2, space="PSUM"))

        # ---- persistent SBUF ----
        qt_sb = [sb.tile([P, S], dt.bfloat16, name=f"qt{i}", tag=f"qt{i}") for i in range(2)]
        kt_sb = [sb.tile([P, S], dt.bfloat16, name=f"kt{i}", tag=f"kt{i}") for i in range(2)]
        v_sb = [sb.tile([P, HPC * VW], dt.bfloat16, name=f"v{i}", tag=f"v{i}") for i in range(16)]
        hot_sb = [sb.tile([P, S], dt.bfloat16, name=f"ho{i}", tag=f"ho{i}") for i in range(2)]
        wo_sb = [sb.tile([P, C], dt.bfloat16, name=f"wo{i}", tag=f"wo{i}") for i in range(2)]
        bqk_sb = sb.tile([P, 4], dt.float32, name="bqk", tag="bqk")
        bv_sb = sb.tile([1, DH], dt.bfloat16, name="bv", tag="bv")
        ones_sb = sb.tile([1, P], dt.bfloat16, name="ones", tag="ones")
        # exp bank A (heads 0, 2): two fresh half-banks
        expA = [sb.tile([P, HB], dt.bfloat16, name=f"expA{i}", tag=f"expA{i}")
                for i in range(2)]
        # exp bank B (heads 1, 3): half 0 fresh; half 1 aliases xt later
        expB0 = sb.tile([P, HB], dt.bfloat16, name="expB0", tag="expB0")
        expB = [expB0, None]  # [1] filled after the last xt reader

        # ---- phase-1 SBUF (xpool) ----
        # xt as ONE tile: col block i*2048..(i+1)*2048 = c-chunk i of X^T.
        # Weights likewise single tiles with c-chunk i at cols i*DH.
        xt_sb = xpool.tile([P, 8 * S], dt.bfloat16, name="xbig", tag="xbig")
        wq_sb = xpool.tile([P, 8 * DH], dt.bfloat16, name="wqb", tag="wqb")
        wk_sb = xpool.tile([P, 8 * DH], dt.bfloat16, name="wkb", tag="wkb")
        wv_sb = xpool.tile([P, 8 * DH], dt.bfloat16, name="wvb", tag="wvb")

        # ---- loads ----
        # One batched 3D descriptor per xt s-quarter (sync queue, arrivals
        # ~3/6/9/12us) and per weight tensor (scalar queue, all by ~7us so
        # the first K/Q units start ~4us in; scalar is free again before
        # the first exp).
        xt_v = xt_sb.rearrange("p (n s) -> p n s", s=S)
        xt_p = xt.rearrange("(n p) s -> p n s", p=P)

        def xt_quarter(q):
            return (xt_v[:, :, q * 512:(q + 1) * 512],
                    xt_p[:, :, q * 512:(q + 1) * 512])

        # sync queue: the four xt quarters in consumption order; scalar
        # queue: wk then wq (the first units' gates) then small tensors.
        for q in range(4):
            o, i = xt_quarter(q)
            nc.sync.dma_start(out=o, in_=i)
        nc.scalar.dma_start(
            out=wk_sb.rearrange("p (n d) -> p n d", d=DH),
            in_=wk.rearrange("(n p) d -> p n d", p=P))
        nc.scalar.dma_start(
            out=wq_sb.rearrange("p (n d) -> p n d", d=DH),
            in_=wq.rearrange("(n p) d -> p n d", p=P))
        nc.scalar.dma_start(out=bqk_sb, in_=bqk[:, :])
        nc.scalar.dma_start(
            out=wv_sb.rearrange("p (n d) -> p n d", d=DH),
            in_=wv.rearrange("(n p) d -> p n d", p=P))
        nc.scalar.dma_start(out=bv_sb, in_=bv[:, :])
        nc.gpsimd.dma_start(out=wo_sb[0], in_=wo_r[0])
        nc.gpsimd.dma_start(out=wo_sb[1], in_=wo_r[1])
        nc.vector.memset(ones_sb, 1.0)
        for t in range(16):
            col = v_sb[t].rearrange("p (h w) -> p h w", w=VW)[:, :, DK:DK + 1]
            nc.vector.memset(col, 1.0)

        def exp_bank(h):
            if h % 2 == 0:
                return expA
            if expB[1] is None:
                # alias the xt bytes (same pool tag + size -> reuse + WAR)
                expB[1] = xpool.tile([P, HB], dt.bfloat16, name="expB1",
                                     tag="xbig")
            return expB

        def qkt_unit(qk, d2, sc):
            """One [128, 512] s-chunk of QT or KT for d-tile d2."""
            dst, w_sb, bias_col = (
                (qt_sb, wq_sb, 0) if qk == 0 else (kt_sb, wk_sb, 2)
            )
            ps = spool.tile([P, 512], dt.float32, name="mm", tag="mm")
            for c8 in range(8):
                nc.tensor.matmul(
                    ps,
                    lhsT=w_sb[:, c8 * DH + d2 * P: c8 * DH + (d2 + 1) * P],
                    rhs=xt_sb[:, c8 * S + sc * 512: c8 * S + (sc + 1) * 512],
                    start=(c8 == 0), stop=(c8 == 7),
                )
            nc.vector.tensor_scalar_add(
                dst[d2][:, sc * 512:(sc + 1) * 512], ps,
                bqk_sb[:, bias_col + d2:bias_col + d2 + 1],
            )

        def sc_tile(h, half, j):
            """Scores+exp tile j of half-bank `half` for head h (t-major:
            flattened col = (t%8)*2048 + s)."""
            c0, w = EXP_TILES[j]
            half_idx = h // 2
            row0 = (h % 2) * DK
            kth, qth = kt_sb[half_idx], qt_sb[half_idx]
            ps = spool.tile([P, w], dt.float32, name="mm", tag="mm")
            for cc in range(w // 512):
                c = c0 + cc * 512
                t = half * 8 + c // S
                s = c % S
                nc.tensor.matmul(
                    ps[:, cc * 512:(cc + 1) * 512],
                    lhsT=kth[row0:row0 + DK, t * P:(t + 1) * P],
                    rhs=qth[row0:row0 + DK, s:s + 512],
                    start=True, stop=True,
                )
            nc.scalar.activation(exp_bank(h)[half][:, c0:c0 + w], ps, AF.Exp)

        def sc3_tile(q, j):
            """Scores+exp tile for head 3, s-major: quarter-block q holds
            cols t*512 for all 16 t at s-quarter q."""
            c0, w = EXP_TILES_Q[j]
            row0 = DK  # head 3 = odd head of d-tile 1
            kth, qth = kt_sb[1], qt_sb[1]
            ps = spool.tile([P, w], dt.float32, name="mm", tag="mm")
            for cc in range(w // 512):
                t = (c0 + cc * 512) // 512
                nc.tensor.matmul(
                    ps[:, cc * 512:(cc + 1) * 512],
                    lhsT=kth[row0:row0 + DK, t * P:(t + 1) * P],
                    rhs=qth[row0:row0 + DK, q * 512:(q + 1) * 512],
                    start=True, stop=True,
                )
            dst = exp_bank(3)[q // 2]
            base = (q % 2) * 8192
            nc.scalar.activation(dst[:, base + c0: base + c0 + w], ps, AF.Exp)

        def vproj_t(t):
            # V tile t: [128, 256] + bias via ones-row; packed [64|1|pad] x4.
            ps = spool.tile([P, DH], dt.float32, name="mm", tag="mm")
            for c8 in range(8):
                nc.tensor.matmul(
                    ps, lhsT=xt_sb[:, c8 * S + t * P: c8 * S + (t + 1) * P],
                    rhs=wv_sb[:, c8 * DH:(c8 + 1) * DH],
                    start=(c8 == 0), stop=False,
                )
            nc.tensor.matmul(ps, lhsT=ones_sb, rhs=bv_sb,
                             start=False, stop=True)
            dst = v_sb[t].rearrange("p (h w) -> p h w", w=VW)[:, :, 0:DK]
            src = ps.rearrange("p (h w) -> p h w", w=DK)
            nc.vector.tensor_copy(dst, src)

        def norm_q(h, q, pv):
            """Normalize attnV s-quarter q of head h out of accumulator pv."""
            half_idx = h // 2
            row0 = (h % 2) * DK
            cs = slice(q * 512, (q + 1) * 512)
            sumtmp = pool2.tile([1, 512], dt.float32, name="sumtmp",
                                tag="sumtmp", bufs=2)
            sr = pool2.tile([1, 512], dt.float32, name="sr", tag="sr", bufs=2)
            rbc = pool2.tile([DK, 512], dt.float32, name="rbc", tag="rbc",
                             bufs=2)
            nc.vector.tensor_copy(sumtmp, pv[DK:DK + 1, :])
            nc.vector.reciprocal_approx_fast(sr, sumtmp)
            nc.gpsimd.partition_broadcast(rbc, sr)
            nc.vector.tensor_mul(
                hot_sb[half_idx][row0:row0 + DK, cs], pv[:DK, :], rbc,
            )

        def attnv_pair_begin(h, pair):
            """attnV s-quarters (2*pair, 2*pair+1) of head h, t-outer over
            two 1-bank accumulators (t-major exp layout, heads 0-2)."""
            pva = vpool.tile([DK + 1, 512], dt.float32, name="pv", tag="pv")
            pvb = vpool.tile([DK + 1, 512], dt.float32, name="pv", tag="pv")
            return (h, pair, pva, pvb)

        def attnv_pair_step(actx, t):
            h, pair, pva, pvb = actx
            eh = exp_bank(h)
            half = t // 8
            base = (t % 8) * S
            for q, pv in ((2 * pair, pva), (2 * pair + 1, pvb)):
                nc.tensor.matmul(
                    pv,
                    lhsT=v_sb[t][:, h * VW:h * VW + DK + 1],
                    rhs=eh[half][:, base + q * 512: base + (q + 1) * 512],
                    start=(t == 0), stop=(t == 15),
                )

        def attnv_pair_end(actx):
            h, pair, pva, pvb = actx
            norm_q(h, 2 * pair, pva)
            norm_q(h, 2 * pair + 1, pvb)

        def attnv3_q_mms(q):
            """attnV s-quarter q of head 3 (s-major layout): rides exp(3)."""
            eh = exp_bank(3)[q // 2]
            base = (q % 2) * 8192
            pv = vpool.tile([DK + 1, 512], dt.float32, name="pv", tag="pv")
            for t in range(16):
                nc.tensor.matmul(
                    pv,
                    lhsT=v_sb[t][:, 3 * VW:3 * VW + DK + 1],
                    rhs=eh[:, base + t * 512: base + (t + 1) * 512],
                    start=(t == 0), stop=(t == 15),
                )
            return pv

        def outproj_sq(q):
            """Output projection for s-quarter q. PSUM from the pv slots
            (scores slots are still draining exp(3)); copies and DMA
            rotate engines; ACT is free at the tail so it helps too."""
            for e in range(8):
                ps = vpool.tile([P, 512], dt.float32, name="pv", tag="pv")
                for d2 in range(2):
                    nc.tensor.matmul(
                        ps,
                        lhsT=wo_sb[d2][:, e * P:(e + 1) * P],
                        rhs=hot_sb[d2][:, q * 512:(q + 1) * 512],
                        start=(d2 == 0), stop=(d2 == 1),
                    )
                st = pool2.tile([P, 512], dt.bfloat16, name="st",
                                tag="st", bufs=3)
                if e % 2 == 0:
                    nc.vector.tensor_copy(st, ps)
                else:
                    nc.scalar.copy(st, ps)
                (nc.sync, nc.gpsimd, nc.scalar)[e % 3].dma_start(
                    out=outT_r[e][:, q * 512:(q + 1) * 512], in_=st)

        # ================= emission =================
        # Four balanced streams: stream h runs head h's score tiles (the
        # food for exp(h) on ACT) interleaved ~1:1 with filler PE work
        # whose deps are already satisfied, so neither engine ever idles
        # long: S0 filler = QKT d2=0 + vproj, S1 = attnV(0) + QKT d2=1,
        # S2 = attnV(1), S3 = attnV(2).  Tail = attnV(3) riding exp(3)
        # s-major + the output projection per s-quarter.

        def interleave(tiles, fillers, lead=2):
            """Emit score tiles and filler thunks, pacing tiles evenly but
            running `lead` tiles ahead so ACT never waits on food."""
            nt, nf = len(tiles), len(fillers)
            ti = fi = 0
            while ti < nt or fi < nf:
                if ti < nt and (fi >= nf or (ti - lead) * nf <= fi * nt):
                    tiles[ti]()
                    ti += 1
                else:
                    fillers[fi]()
                    fi += 1

        def T(h, half, j):
            return lambda: sc_tile(h, half, j)

        def T3(q, j):
            return lambda: sc3_tile(q, j)

        def attn_steps(actx, ts):
            return [lambda t=t: attnv_pair_step(actx, t) for t in ts]

        # ---- stream 0 ----
        # hand-ordered prologue so every score tile follows its K/Q units
        # (PE executes in issue order: a tile emitted before its unit would
        # head-block the PE stream forever).
        qkt_unit(1, 0, 0)               # K d2=0 s 0:512
        qkt_unit(0, 0, 0)               # Q s 0:512
        qkt_unit(0, 0, 1)
        qkt_unit(0, 0, 2)
        sc_tile(0, 0, 0)
        qkt_unit(0, 0, 3)
        sc_tile(0, 0, 1)
        qkt_unit(1, 0, 1)
        sc_tile(0, 0, 2)
        sc_tile(0, 0, 3)
        qkt_unit(1, 0, 2)
        sc_tile(0, 0, 4)
        sc_tile(0, 0, 5)
        qkt_unit(1, 0, 3)
        s0_tiles = ([T(0, 0, j) for j in range(6, 11)]
                    + [T(0, 1, j) for j in range(11)])
        interleave(s0_tiles, [lambda t=t: vproj_t(t) for t in range(16)])

        # ---- stream 1 ----
        # fresh half first (no xt dep) against attnV(0) pair 0 + QKT d2=1;
        # the aliased half follows once every xt reader is emitted.
        a0p0 = attnv_pair_begin(0, 0)
        s1_fill_a = (attn_steps(a0p0, range(8))
                     + [lambda qk=qk, sc=sc: qkt_unit(qk, 1, sc)
                        for qk in (1, 0) for sc in range(4)])
        interleave([T(1, 0, j) for j in range(11)], s1_fill_a)
        s1_post = [T(1, 1, j) for j in range(11)]
        interleave(s1_post[:5], attn_steps(a0p0, range(8, 16)))
        attnv_pair_end(a0p0)
        a0p1 = attnv_pair_begin(0, 1)
        interleave(s1_post[5:], attn_steps(a0p1, range(16)))
        attnv_pair_end(a0p1)

        # ---- streams 2 and 3 ----
        for h, tiles in (
            (1, [T(2, half, j) for half in range(2) for j in range(11)]),
            (2, [T3(q, j) for q in range(4) for j in range(6)]),
        ):
            ap0 = attnv_pair_begin(h, 0)
            interleave(tiles[:len(tiles) // 2], attn_steps(ap0, range(16)))
            attnv_pair_end(ap0)
            ap1 = attnv_pair_begin(h, 1)
            interleave(tiles[len(tiles) // 2:], attn_steps(ap1, range(16)))
            attnv_pair_end(ap1)

        # ---- tail: head-3 attnV + outproj per s-quarter, staggered so the
        # PE never waits on a norm chain (outproj(q) needs only quarter q's
        # norm, which completed during the previous PE block), and every
        # pv-slot reuse follows its reader in emission order.
        pv0 = attnv3_q_mms(0)
        norm_q(3, 0, pv0)
        pv1 = attnv3_q_mms(1)
        norm_q(3, 1, pv1)
        outproj_sq(0)
        pv2 = attnv3_q_mms(2)
        norm_q(3, 2, pv2)
        outproj_sq(1)
        pv3 = attnv3_q_mms(3)
        norm_q(3, 3, pv3)
        outproj_sq(2)
        outproj_sq(3)

    nc.compile()
    return nc


def _get_program():
    if "nc" not in _CACHE:
        _CACHE["nc"] = _build_program()
    return _CACHE["nc"]


def _shard_inputs(input, W_qkv, b_qkv, W_out):
    """Build the 8 per-core input maps (host-side shard + transpose + cast)."""
    in_maps = []
    xt_by_b = [
        np.ascontiguousarray(input[b].T).astype(BF16) for b in range(2)
    ]
    for core in range(N_CORES):
        b, g = divmod(core, HPC)
        cols = slice(g * DH, (g + 1) * DH)
        bq = (b_qkv[g * DH:(g + 1) * DH] / 8.0).astype(np.float32)
        bk = b_qkv[C + g * DH:C + (g + 1) * DH].astype(np.float32)
        bqk = np.stack([bq[:P], bq[P:], bk[:P], bk[P:]], axis=1)
        in_maps.append({
            "xt": xt_by_b[b],
            "wq": np.ascontiguousarray(W_qkv[:, cols] * 0.125).astype(BF16),
            "wk": np.ascontiguousarray(W_qkv[:, C:2 * C][:, cols]).astype(BF16),
            "wv": np.ascontiguousarray(W_qkv[:, 2 * C:][:, cols]).astype(BF16),
            "wo": np.ascontiguousarray(W_out[g * DH:(g + 1) * DH, :]).astype(BF16),
            "bqk": np.ascontiguousarray(bqk, dtype=np.float32),
            "bv": b_qkv[2 * C + g * DH:2 * C + (g + 1) * DH]
                  .astype(BF16).reshape(1, DH),
        })
    return in_maps


def kernel(input, W_qkv, b_qkv, W_out):
    from concourse.bass_utils import run_bass_kernel_spmd

    nc = _get_program()
    in_maps = _shard_inputs(
        np.asarray(input), np.asarray(W_qkv), np.asarray(b_qkv),
        np.asarray(W_out),
    )
    res = run_bass_kernel_spmd(nc, in_maps, core_ids=list(range(N_CORES)))
    out = np.zeros((2, S, C), dtype=np.float32)
    for core in range(N_CORES):
        b = core // HPC
        out[b] += np.asarray(res.results[core]["outT"]).astype(np.float32).T
    return out
